# revision 54
# baseline (speedup 1.0000x reference)
"""Trainium2 Bass kernel for nn_AttentionPoolingTemporalEncoder.

Strategy (data-parallel over batch, 8 cores, 4 batch rows each):
  device:  h8 = relu(x @ (64*Wp))/256      (fp8 DoubleRow matmuls; h8 is the
                                            ONLY materialization of h, fp8 at
                                            true_h/4 scale, one vector op)
           scores = h8 @ (4*(Wk @ qh)/sqrt(D))  (bk shifts cancel in softmax)
           p = exp(scores + maskbias)/16   (no running max; scores are O(5))
           U[h,:] = sum_s p[s,h] * h8[s,:] ; Z[h] = sum_s p[s,h]
  host:    pooled = 4*(U/Z) @ Wv (+bv) per head; @Wo+bo; @W2+b2; LayerNorm.

v18 design notes (what mattered, in order):
- x is cast to fp8 on the HOST and packed pre-transposed per 1MB
  DRAM-contiguous chunk [BL, k, 128in, C4, 2, 1024s] — a chunk streams at
  8KB/partition-line, and the projection uses it as the DR stationary.
- scores need h^T: ONE XBAR transpose per 8-s-tile batch of h8 VIEWED AS
  uint16 (XBAR has no 1-byte mode; adjacent e-pairs ride as one element).
  The score matmuls undo the pairing with stride-2 fp8 stationary APs and
  host-deinterleaved fp8 wkq (wkq8[c,j,p,:] = wkq[256c+2p+j,:]). This
  halves transpose DMA bytes vs bf16 AND removes every cast op.
- Queue placement is the whole game: x chunks issue from the scalar HWDGE
  queue (chunks 1,3 from sync while it is transpose-light), transposes from
  sync, exp + drains on scalar, relu on vector, mb loads on gpsimd. SWDGE
  (gpsimd) data movement anywhere hot loses ~10-30us to SDMA contention.
- The 8 DMAHW completion lanes are shared round-robin in EMISSION order:
  a prefetch burst makes early transposes' lanes alias in-flight 1MB
  chunks (false deps that stall scores ~7us/batch) — hence the RAMPED
  prefetch (2ci+3, capped ci+9) with deep xp buffering.
- Tails (scores+exp) trail the projection by 4 batches; U/Z trail one
  more. ~56 dependency-free warm-up matmuls run during the DMA fill so
  the first projections start with HAM at 2.4GHz.
- History: 272us baseline -> ~197-204us. Failed experiments (reverted):
  SWDGE cast-DMA for h8 (contends with transposes), p-export for host-Z
  (observer placement stalls whichever queue Tile parks it on), 512KB
  chunks (2x issue/observer overhead), partition-split mid-kernel chunks
  (NaN: multi-writer tile dep tracking), split transposes, pending!=4-5.
"""

import sys
import threading

import numpy as np

sys.path.insert(0, "/opt/trn_rl_repo")

from contextlib import ExitStack

import concourse.tile as tile
from concourse import bacc, mybir
from concourse.bass_utils import run_bass_kernel_spmd


def _ensure_axon_ntff_hook_module():
    """Some images lack ``antenv.axon_hooks``; concourse imports it
    unconditionally when tracing is requested (e.g. via BASS_TRACE).
    Provide a minimal stand-in so that path degrades to no-trace
    instead of crashing."""
    try:
        from antenv import axon_hooks  # noqa: F401

        return
    except ImportError:
        pass
    import types

    mod = types.ModuleType("antenv.axon_hooks")
    mod._hook = None

    def set_axon_ntff_profile_hook(h):
        mod._hook = h

    def get_axon_ntff_profile_hook():
        return mod._hook

    mod.set_axon_ntff_profile_hook = set_axon_ntff_profile_hook
    mod.get_axon_ntff_profile_hook = get_axon_ntff_profile_hook
    sys.modules["antenv.axon_hooks"] = mod
    try:
        import antenv

        antenv.axon_hooks = mod
    except ImportError:
        pass


_ensure_axon_ntff_hook_module()

# Problem sizes (hardcoded per spec)
B, S, IN_DIM, E, H = 32, 4096, 1024, 512, 8
D = E // H
NCORES = 8
P = 128
WP_SCALE = 64.0  # Wp pre-scaled into fp8's sweet spot; relu is homogeneous
TB = 8           # s-tiles per transpose batch / U group

_nc_cache = {}
_nc_lock = threading.Lock()


def build_nc(BL=B // NCORES, S_=S, I_=IN_DIM, has_bp=False):
    """Build + compile the per-core Bass program."""
    key = (BL, S_, I_, has_bp)
    with _nc_lock:
        if key in _nc_cache:
            return _nc_cache[key]

    C4 = I_ // 256      # 256-deep DoubleRow contraction chunks
    EC = E // P         # embed-dim chunks
    S_TILES = S_ // P   # s-tiles per batch row
    SC = 1024           # s-positions per x chunk (1MB, DRAM-contiguous)
    NCH = S_ // SC

    # x chunk list: (row, chunk_idx). Each chunk is a fully contiguous 1MB
    # DRAM block (8KB per partition line) so the transfer runs at full HBM
    # bandwidth instead of descriptor-overhead-bound.
    chunks = [(bb, k) for bb in range(BL) for k in range(NCH)]

    f32 = mybir.dt.float32
    bf16 = mybir.dt.bfloat16
    u16 = mybir.dt.uint16
    fp8 = mybir.dt.float8e4
    EXP = mybir.ActivationFunctionType.Exp
    RELU = mybir.ActivationFunctionType.Relu
    DR = mybir.MatmulPerfMode.DoubleRow

    nc = bacc.Bacc(
        "TRN2",
        target_bir_lowering=False,
        debug=False,
        enable_asserts=False,
        num_devices=NCORES,
    )

    NB = BL * S_TILES // TB  # transpose batches per core

    xt = nc.dram_tensor(
        "xt", [BL, NCH, P, C4, 2, SC], fp8, kind="ExternalInput"
    ).ap()
    wp = nc.dram_tensor("wp", [P, C4, 2, E], fp8, kind="ExternalInput").ap()
    wkq = nc.dram_tensor("wkq", [P, 2, 2, H], fp8, kind="ExternalInput").ap()
    mb = nc.dram_tensor("mb", [BL, P, S_TILES], f32, kind="ExternalInput").ap()
    if has_bp:
        bp_d = nc.dram_tensor("bp", [1, E], bf16, kind="ExternalInput").ap()
    u_out = nc.dram_tensor("u_out", [BL, P, E], f32, kind="ExternalOutput").ap()
    z_out = nc.dram_tensor("z_out", [BL, H, 1], f32, kind="ExternalOutput").ap()

    with tile.TileContext(nc) as tc, ExitStack() as ctx:
        const = ctx.enter_context(tc.tile_pool(name="const", bufs=1))
        xp = ctx.enter_context(tc.tile_pool(name="xp", bufs=10))
        h8p = ctx.enter_context(tc.tile_pool(name="h8p", bufs=10))
        htp = ctx.enter_context(tc.tile_pool(name="htp", bufs=8))
        pp = ctx.enter_context(tc.tile_pool(name="pp", bufs=9))
        mbp = ctx.enter_context(tc.tile_pool(name="mbp", bufs=4))
        uzp = ctx.enter_context(tc.tile_pool(name="uzp", bufs=2))
        ps_h = ctx.enter_context(tc.tile_pool(name="ps_h", bufs=3, space="PSUM"))
        ps_s = ctx.enter_context(tc.tile_pool(name="ps_s", bufs=2, space="PSUM"))
        ps_u = ctx.enter_context(tc.tile_pool(name="ps_u", bufs=2, space="PSUM"))
        ps_z = ctx.enter_context(tc.tile_pool(name="ps_z", bufs=1, space="PSUM"))

        def load_chunk(idx):
            bb, k = chunks[idx]
            xt_c = xp.tile([P, C4, 2, SC], fp8, tag="xchunk")
            if idx == 0:
                # split the startup-critical first chunk by PARTITION rows
                # (each half stays 8KB-element contiguous) across both
                # HWDGE queues to halve the pipeline fill
                nc.scalar.dma_start(xt_c[0:64], xt[bb, k, 0:64])
                nc.sync.dma_start(xt_c[64:P], xt[bb, k, 64:P])
                return xt_c
            if idx in (1, 3):
                # two early chunks ride the sync ring while it is still
                # transpose-light, doubling early x bandwidth while the
                # prefetch lead builds
                nc.sync.dma_start(xt_c[:], xt[bb, k])
                return xt_c
            nc.scalar.dma_start(xt_c[:], xt[bb, k])
            return xt_c

        # Startup order: first x chunk halves, then wp, then one more chunk
        # — a small initial burst so the startup-critical transfers aren't
        # stuck behind megabytes of packet-interleaved prefetch.
        bufq = [load_chunk(0)]
        wp_sb = const.tile([P, C4, 2, E], fp8)
        nc.sync.dma_start(wp_sb[:], wp[:])
        bufq.append(load_chunk(1))
        next_load = len(bufq)

        wkq_sb = const.tile([P, 2, 2, H], fp8)
        nc.sync.dma_start(wkq_sb[:], wkq[:])
        ones_t = const.tile([P, 2, 16], fp8)
        nc.gpsimd.memset(ones_t[:], 1.0)
        if has_bp:
            ones_row = const.tile([1, P], bf16)
            nc.gpsimd.memset(ones_row[:], 1.0)
            bp_sb = const.tile([1, E], bf16)
            nc.sync.dma_start(bp_sb[:], bp_d[:])

        # HAM warm-up: dependency-free N=512 dummy matmuls keep the PE
        # continuously active through the DMA pipeline fill (~7us), so the
        # first real projection matmuls run at 2.4 GHz instead of 1.2 GHz
        dum_sb = const.tile([P, E], fp8)
        nc.gpsimd.memset(dum_sb[:], 1.0)
        warm_ps = ps_h.tile([P, E], f32, tag="h_ps")
        for _ in range(32):
            nc.tensor.matmul(
                warm_ps[0:1, :], ones_t[:, 0, 0:1], dum_sb[:],
                start=True, stop=True,
            )

        mb_tiles = {}

        def load_mb(bb):
            mb_t = mbp.tile([P, S_TILES], f32)
            nc.gpsimd.dma_start(mb_t[:], mb[bb])
            mb_tiles[bb] = mb_t

        load_mb(0)
        if BL > 1:
            load_mb(1)

        row_state = {}  # b -> (u_ps, z_ps)
        COPY = mybir.ActivationFunctionType.Copy
        uz_queue = []     # stage-B work: (b, t0, h_se_b, p_b)
        drain_queue = []  # row drains, delayed >= 1 batch so the scalar
                          # copies never wait on in-flight U matmuls

        def emit_drains():
            while drain_queue:
                b_, u_ps, z_ps = drain_queue.pop(0)
                u_sb = uzp.tile([P, E], f32, tag="u_sb")
                z_sb = uzp.tile([H, 1], f32, tag="z_sb")
                nc.scalar.activation(u_sb[:], u_ps[:], COPY)
                nc.scalar.activation(z_sb[:], z_ps[:, 0:1], COPY)
                nc.scalar.dma_start(u_out[b_], u_sb[:])
                nc.scalar.dma_start(z_out[b_], z_sb[:])

        def emit_uz(work):
            """Stage B: U/Z matmuls, one transpose batch behind stage A
            so the exps have a full batch period to complete."""
            b_, t0_, h8_b, p_b = work
            u_ps, z_ps = row_state[b_]
            # U as 2 fp8 DoubleRow matmuls (adjacent s-tiles paired into
            # the K dimension), both accumulating into the same region
            # (DR forbids non-zero destination col groups)
            for j in range(TB // 2):
                nc.tensor.matmul(
                    u_ps[0:H, :],
                    p_b[:, 2 * j : 2 * j + 2, 0:H],
                    h8_b[:, 2 * j : 2 * j + 2, :],
                    start=(t0_ == 0 and j == 0),
                    stop=(t0_ + TB == S_TILES and j == TB // 2 - 1),
                    skip_group_check=True,
                    perf_mode=DR,
                )
            # Z as 2 fp8 DoubleRow matmuls over the same p pairs
            for j in range(TB // 2):
                nc.tensor.matmul(
                    z_ps[:],
                    p_b[:, 2 * j : 2 * j + 2, 0:H],
                    ones_t[:, :, 0:2],
                    start=(t0_ == 0 and j == 0),
                    stop=(t0_ + TB == S_TILES and j == TB // 2 - 1),
                    skip_group_check=True,
                    perf_mode=DR,
                )
            if t0_ + TB == S_TILES:
                drain_queue.append((b_, u_ps, z_ps))
                del row_state[b_]

        def emit_tails(pend):
            """Stage A for one transpose batch: scores + exp; then stage
            B (U/Z) for the previous batch and any due row drains."""
            b_, t0_, h8_b, ht_b = pend
            emit_drains()
            if b_ not in row_state:
                u_ps = ps_u.tile([P, E], f32)
                z_ps = ps_z.tile([H, 2], f32)
                row_state[b_] = (u_ps, z_ps)
            mb_t = mb_tiles[b_]
            # p padded to 16 cols: the DoubleRow weights AP needs a
            # 16-byte-aligned stride on the pair axis
            p_b = pp.tile([P, TB, 16], fp8)
            ht_v = ht_b[:].bitcast(fp8).rearrange(
                "p g (s two) -> p g two s", two=2
            )  # [P, TB*2, 2, 128]; (g=2c+?, two=j) selects e = 256c+2*e2+j
            for tt in range(TB):
                t_ = t0_ + tt
                # scores[s,h] = sum_e h[s,e] wkq[e,h]
                sc_ps = ps_s.tile([P, H], f32)
                kk = 0
                for c in range(2):
                    for jj in range(2):
                        nc.tensor.matmul(
                            sc_ps[:],
                            ht_v[:, tt * 2 + c, jj, :],
                            wkq_sb[:, c, jj, :],
                            start=(kk == 0),
                            stop=(kk == 3),
                        )
                        kk += 1
                # p = exp(scores + maskbias)/16; maskbias = -ln16 kept,
                # -1e4 masked (the 1/16 keeps fp8 p well inside e4m3
                # range; U/Z is scale-invariant so the host is unchanged)
                nc.scalar.activation(
                    p_b[:, tt, 0:H], sc_ps[:], EXP, bias=mb_t[:, t_ : t_ + 1]
                )
            uz_queue.append((b_, t0_, h8_b, p_b))
            if len(uz_queue) > 1:
                emit_uz(uz_queue.pop(0))

        pending = []
        for ci, (b, kch) in enumerate(chunks):
            s0, sl = kch * SC, SC
            x_sb = bufq.pop(0)
            # prefetch EARLY in program order so the transfers get multiple
            # chunk-periods of lead time — but RAMPED (not a burst): DMAHW
            # completion lanes are shared 8-wide round-robin in emission
            # order, so a startup burst makes the first transposes' lanes
            # alias still-in-flight megabyte chunks (false dependencies
            # that stall the score matmuls ~7us per early batch)
            while next_load < len(chunks) and next_load <= min(
                ci + 9, 2 * ci + 2
            ):
                bufq.append(load_chunk(next_load))
                next_load += 1
            for j in range(sl // (TB * P)):
                # --- projection for TB s-tiles ---
                h8_b = h8p.tile([P, TB, E], fp8, tag="h8")
                for tt in range(TB):
                    ts = (j * TB + tt) * P
                    h_ps = ps_h.tile([P, E], f32)
                    for c in range(C4):
                        nc.tensor.matmul(
                            h_ps[:],
                            x_sb[:, c, :, ts : ts + P],
                            wp_sb[:, c, :, :],
                            start=(c == 0),
                            stop=(c == C4 - 1) and not has_bp,
                            perf_mode=DR,
                        )
                    if has_bp:
                        nc.tensor.matmul(
                            h_ps[:], ones_row[:], bp_sb[:], start=False, stop=True
                        )
                    # relu + 1/256 scale in ONE vector op, straight to
                    # fp8: h8 = h_true/4 is the ONLY materialization of h —
                    # it feeds the U matmuls, and (bitcast to u16 pairs) the
                    # XBAR transpose for the score matmuls
                    nc.vector.tensor_scalar(
                        h8_b[:, tt, :], h_ps[:], 0.0, 1.0 / 256.0,
                        mybir.AluOpType.max, mybir.AluOpType.mult,
                    )
                # --- one batched SBUF->SBUF XBAR transpose for TB tiles.
                # Sync queue only: concurrent transposes from both HWDGE
                # queues race on the shared XBAR and corrupt data
                # (observed as nondeterministic output error) ---
                # one u16-viewed XBAR transpose per batch: adjacent
                # fp8 e-pairs ride as single 2-byte elements (the XBAR
                # does not support 1-byte dtypes), halving transpose bytes
                # vs a bf16 h. The score matmuls undo the pairing with
                # stride-2 stationary APs + host-deinterleaved wkq.
                ht_b = htp.tile([P, TB * 2, P], u16, tag="ht")
                nc.sync.dma_start_transpose(
                    ht_b[:], h8_b[:].bitcast(u16)
                )
                pending.append((b, (s0 // P) + j * TB, h8_b, ht_b))
                if len(pending) > 4:
                    emit_tails(pending.pop(0))
            if s0 == 0 and b + 2 < BL:
                load_mb(b + 2)
        while pending:
            emit_tails(pending.pop(0))
        while uz_queue:
            emit_uz(uz_queue.pop(0))
        emit_drains()

    nc.compile()
    with _nc_lock:
        _nc_cache[key] = nc
    return nc


def prepare_core_inputs(x, mask, Wp8, wkq8, bp=None):
    """Host-side packing for ONE core's shard."""
    import ml_dtypes

    fp8 = ml_dtypes.float8_e4m3
    BL_, S_, I_ = x.shape
    C4 = I_ // 256
    SC = 1024
    # xt[b, k, ki, c, ko, s] = x[b, SC*k + s, 256c + 128ko + ki]
    # (chunk-contiguous in DRAM: one chunk = one sequential 1MB block)
    x8 = x.astype(fp8)
    xt = np.ascontiguousarray(
        x8.reshape(BL_, S_ // SC, SC, C4, 2, P).transpose(0, 1, 5, 3, 4, 2)
    )
    # additive mask bias packed [BL, P, S_TILES]: -ln16 where kept (keeps
    # fp8 p inside e4m3 range; U/Z is invariant to the scale), -1e4 where
    # masked (exp underflows to exactly 0)
    mb = np.ascontiguousarray(
        ((mask.astype(np.float32) - 1.0) * 1.0e4 - np.log(16.0))
        .reshape(BL_, S_ // P, P)
        .transpose(0, 2, 1)
    ).astype(np.float32)
    m = {"xt": xt, "wp": Wp8, "wkq": wkq8, "mb": mb}
    if bp is not None:
        import ml_dtypes as md

        m["bp"] = (np.asarray(bp) * WP_SCALE).astype(md.bfloat16).reshape(1, E)
    return m


def kernel(
    x, mask, query, Wp, bp, Wq, bq, Wk, bk, Wv, bv, Wo, bo, W2, b2, gamma, beta,
    _trace=False,
):
    import ml_dtypes

    x = np.asarray(x)
    mask = np.asarray(mask)
    BL = B // NCORES

    # Host-side folds (all tiny)
    qh = (np.asarray(query, np.float64) @ np.asarray(Wq, np.float64)
          + np.asarray(bq, np.float64)).reshape(H, D)
    # h8 on-chip is at (1/4) the true h scale, so the folded score
    # weights carry a x4 factor; packed fp8, deinterleaved to match the
    # u16 pair-transpose: wkq8[c, j, p, :] = wkq[256c + 2p + j, :]
    wkq_scaled = np.einsum(
        "ehd,hd->eh",
        np.asarray(Wk, np.float64).reshape(E, H, D),
        qh,
    ) / np.sqrt(D) * 4.0
    wkq8 = np.ascontiguousarray(
        wkq_scaled.astype(np.float32)
        .reshape(2, 128, 2, H)
        .transpose(1, 0, 2, 3)
    ).astype(ml_dtypes.float8_e4m3)
    C4 = IN_DIM // 256
    Wp8 = np.ascontiguousarray(
        (np.asarray(Wp, np.float32) * WP_SCALE)
        .reshape(C4, 2, P, E)
        .transpose(2, 0, 1, 3)
    ).astype(ml_dtypes.float8_e4m3)

    has_bp = bool(np.any(np.asarray(bp)))
    nc = build_nc(has_bp=has_bp)

    in_maps = []
    for c in range(NCORES):
        sl = slice(c * BL, (c + 1) * BL)
        in_maps.append(
            prepare_core_inputs(
                x[sl], mask[sl], Wp8, wkq8,
                bp=np.asarray(bp) if has_bp else None,
            )
        )

    res = run_bass_kernel_spmd(
        nc, in_maps, core_ids=list(range(NCORES)), trace=_trace
    )
    u_raw = np.concatenate([r["u_out"] for r in res.results], axis=0)  # (B, P, E)
    z_raw = np.concatenate([r["z_out"] for r in res.results], axis=0)  # (B, H, 1)
    U = u_raw[:, 0:H, :].astype(np.float64)  # (B, H, E); rest is garbage
    Z = z_raw.astype(np.float64)  # (B, H, 1)

    # Host epilogue in float64 (the fp8 U path carries h at 1/4 true scale)
    pooledH = U / (Z * 0.25)  # (B, H, E)
    Wv64 = np.asarray(Wv, np.float64).reshape(E, H, D)
    pooled = np.einsum("bhe,ehd->bhd", pooledH, Wv64).reshape(B, E)
    pooled += np.asarray(bv, np.float64)
    pooled = pooled @ np.asarray(Wo, np.float64) + np.asarray(bo, np.float64)
    out = pooled @ np.asarray(W2, np.float64) + np.asarray(b2, np.float64)
    mu = out.mean(-1, keepdims=True)
    var = out.var(-1, keepdims=True)
    out = (out - mu) / np.sqrt(var + 1e-5) * np.asarray(gamma, np.float64) + np.asarray(
        beta, np.float64
    )
    out_f32 = out.astype(np.float32)
    if _trace:
        return out_f32, res
    return out_f32



# revision 55
# speedup vs baseline: 1.1243x; 1.1243x over previous
"""Trainium2 Bass kernel for nn_AttentionPoolingTemporalEncoder.

Strategy (data-parallel over batch, 8 cores, 4 batch rows each):
  device:  h8 = relu(x @ (64*Wp))/256      (fp8 DoubleRow matmuls; h8 is the
                                            ONLY materialization of h, fp8 at
                                            true_h/4 scale, one vector op)
           scores = h8 @ (4*(Wk @ qh)/sqrt(D))  (bk shifts cancel in softmax)
           p = exp(scores + maskbias)/16   (no running max; scores are O(5))
           U[h,:] = sum_s p[s,h] * h8[s,:] ; Z[h] = sum_s p[s,h]
  host:    pooled = 4*(U/Z) @ Wv (+bv) per head; @Wo+bo; @W2+b2; LayerNorm.

v18 design notes (what mattered, in order):
- x is cast to fp8 on the HOST and packed pre-transposed per 1MB
  DRAM-contiguous chunk [BL, k, 128in, C4, 2, 1024s] — a chunk streams at
  8KB/partition-line, and the projection uses it as the DR stationary.
- scores need h^T: ONE XBAR transpose per 8-s-tile batch of h8 VIEWED AS
  uint16 (XBAR has no 1-byte mode; adjacent e-pairs ride as one element).
  The score matmuls undo the pairing with stride-2 fp8 stationary APs and
  host-deinterleaved fp8 wkq (wkq8[c,j,p,:] = wkq[256c+2p+j,:]). This
  halves transpose DMA bytes vs bf16 AND removes every cast op.
- Queue placement is the whole game: x chunks issue from the scalar HWDGE
  queue (chunks 1,3 from sync while it is transpose-light), transposes from
  sync, exp + drains on scalar, relu on vector, mb loads on gpsimd. SWDGE
  (gpsimd) data movement anywhere hot loses ~10-30us to SDMA contention.
- The 8 DMAHW completion lanes are shared round-robin in EMISSION order:
  a prefetch burst makes early transposes' lanes alias in-flight 1MB
  chunks (false deps that stall scores ~7us/batch) — hence the RAMPED
  prefetch (2ci+3, capped ci+9) with deep xp buffering.
- Tails (scores+exp) trail the projection by 4 batches; U/Z trail one
  more. ~56 dependency-free warm-up matmuls run during the DMA fill so
  the first projections start with HAM at 2.4GHz.
- History: 272us baseline -> ~197-204us. Failed experiments (reverted):
  SWDGE cast-DMA for h8 (contends with transposes), p-export for host-Z
  (observer placement stalls whichever queue Tile parks it on), 512KB
  chunks (2x issue/observer overhead), partition-split mid-kernel chunks
  (NaN: multi-writer tile dep tracking), split transposes, pending!=4-5.
"""

import sys
import threading

import numpy as np

sys.path.insert(0, "/opt/trn_rl_repo")

from contextlib import ExitStack

import concourse.tile as tile
from concourse import bacc, mybir
from concourse.bass_utils import run_bass_kernel_spmd


def _ensure_axon_ntff_hook_module():
    """Some images lack ``antenv.axon_hooks``; concourse imports it
    unconditionally when tracing is requested (e.g. via BASS_TRACE).
    Provide a minimal stand-in so that path degrades to no-trace
    instead of crashing."""
    try:
        from antenv import axon_hooks  # noqa: F401

        return
    except ImportError:
        pass
    import types

    mod = types.ModuleType("antenv.axon_hooks")
    mod._hook = None

    def set_axon_ntff_profile_hook(h):
        mod._hook = h

    def get_axon_ntff_profile_hook():
        return mod._hook

    mod.set_axon_ntff_profile_hook = set_axon_ntff_profile_hook
    mod.get_axon_ntff_profile_hook = get_axon_ntff_profile_hook
    sys.modules["antenv.axon_hooks"] = mod
    try:
        import antenv

        antenv.axon_hooks = mod
    except ImportError:
        pass


_ensure_axon_ntff_hook_module()

# Problem sizes (hardcoded per spec)
B, S, IN_DIM, E, H = 32, 4096, 1024, 512, 8
D = E // H
NCORES = 8
P = 128
WP_SCALE = 64.0  # Wp pre-scaled into fp8's sweet spot; relu is homogeneous
TB = 8           # s-tiles per transpose batch / U group

_nc_cache = {}
_nc_lock = threading.Lock()


def build_nc(BL=B // NCORES, S_=S, I_=IN_DIM, has_bp=False):
    """Build + compile the per-core Bass program."""
    key = (BL, S_, I_, has_bp)
    with _nc_lock:
        if key in _nc_cache:
            return _nc_cache[key]

    C4 = I_ // 256      # 256-deep DoubleRow contraction chunks
    EC = E // P         # embed-dim chunks
    S_TILES = S_ // P   # s-tiles per batch row
    SC = 1024           # s-positions per x chunk (1MB, DRAM-contiguous)
    NCH = S_ // SC

    # x chunk list: (row, chunk_idx). Each chunk is a fully contiguous 1MB
    # DRAM block (8KB per partition line) so the transfer runs at full HBM
    # bandwidth instead of descriptor-overhead-bound.
    chunks = [(bb, k) for bb in range(BL) for k in range(NCH)]

    f32 = mybir.dt.float32
    bf16 = mybir.dt.bfloat16
    u16 = mybir.dt.uint16
    fp8 = mybir.dt.float8e4
    EXP = mybir.ActivationFunctionType.Exp
    RELU = mybir.ActivationFunctionType.Relu
    DR = mybir.MatmulPerfMode.DoubleRow

    nc = bacc.Bacc(
        "TRN2",
        target_bir_lowering=False,
        debug=False,
        enable_asserts=False,
        num_devices=NCORES,
    )

    NB = BL * S_TILES // TB  # transpose batches per core

    xt = nc.dram_tensor(
        "xt", [BL, NCH, P, C4, 2, SC], fp8, kind="ExternalInput"
    ).ap()
    wp = nc.dram_tensor("wp", [P, C4, 2, E], fp8, kind="ExternalInput").ap()
    wkq = nc.dram_tensor("wkq", [2, 2, P, H], fp8, kind="ExternalInput").ap()
    mb = nc.dram_tensor("mb", [BL, P, S_TILES], f32, kind="ExternalInput").ap()
    if has_bp:
        bp_d = nc.dram_tensor("bp", [1, E], bf16, kind="ExternalInput").ap()
    u_out = nc.dram_tensor("u_out", [BL, P, E], f32, kind="ExternalOutput").ap()
    z_out = nc.dram_tensor("z_out", [BL, H, 1], f32, kind="ExternalOutput").ap()

    with tile.TileContext(nc) as tc, ExitStack() as ctx:
        const = ctx.enter_context(tc.tile_pool(name="const", bufs=1))
        xp = ctx.enter_context(tc.tile_pool(name="xp", bufs=10))
        h8p = ctx.enter_context(tc.tile_pool(name="h8p", bufs=10))
        htp = ctx.enter_context(tc.tile_pool(name="htp", bufs=8))
        pp = ctx.enter_context(tc.tile_pool(name="pp", bufs=9))
        mbp = ctx.enter_context(tc.tile_pool(name="mbp", bufs=4))
        uzp = ctx.enter_context(tc.tile_pool(name="uzp", bufs=2))
        ps_h = ctx.enter_context(tc.tile_pool(name="ps_h", bufs=3, space="PSUM"))
        ps_s = ctx.enter_context(tc.tile_pool(name="ps_s", bufs=2, space="PSUM"))
        ps_u = ctx.enter_context(tc.tile_pool(name="ps_u", bufs=2, space="PSUM"))
        ps_z = ctx.enter_context(tc.tile_pool(name="ps_z", bufs=1, space="PSUM"))

        def load_chunk(idx):
            bb, k = chunks[idx]
            xt_c = xp.tile([P, C4, 2, SC], fp8, tag="xchunk")
            if idx == 0:
                # split the startup-critical first chunk by PARTITION rows
                # (each half stays 8KB-element contiguous) across both
                # HWDGE queues to halve the pipeline fill
                nc.scalar.dma_start(xt_c[0:64], xt[bb, k, 0:64])
                nc.sync.dma_start(xt_c[64:P], xt[bb, k, 64:P])
                return xt_c
            if idx in (1, 3):
                # two early chunks ride the sync ring while it is still
                # transpose-light, doubling early x bandwidth while the
                # prefetch lead builds
                nc.sync.dma_start(xt_c[:], xt[bb, k])
                return xt_c
            nc.scalar.dma_start(xt_c[:], xt[bb, k])
            return xt_c

        # Startup order: first x chunk halves, then wp, then one more chunk
        # — a small initial burst so the startup-critical transfers aren't
        # stuck behind megabytes of packet-interleaved prefetch.
        bufq = [load_chunk(0)]
        wp_sb = const.tile([P, C4, 2, E], fp8)
        nc.sync.dma_start(wp_sb[:], wp[:])
        bufq.append(load_chunk(1))
        next_load = len(bufq)

        wkq_sb = const.tile([P, 2, 2, H], fp8)
        nc.sync.dma_start(wkq_sb[:], wkq.rearrange("c j p h -> p c j h"))
        ones_t = const.tile([P, 2, 16], fp8)
        nc.gpsimd.memset(ones_t[:], 1.0)
        if has_bp:
            ones_row = const.tile([1, P], bf16)
            nc.gpsimd.memset(ones_row[:], 1.0)
            bp_sb = const.tile([1, E], bf16)
            nc.sync.dma_start(bp_sb[:], bp_d[:])

        # HAM warm-up: dependency-free dummy matmuls fill the PE's
        # activity window during the DMA pipeline fill, so the first real
        # projection matmuls run at 2.4 GHz instead of the cold 1.2 GHz
        warm_ps = ps_h.tile([P, E], f32, tag="h_ps")
        for _ in range(56):
            nc.tensor.matmul(
                warm_ps[0:1, 0:2], ones_t[:, 0, 0:1], ones_t[:, 0, 0:2],
                start=True, stop=True,
            )

        mb_tiles = {}

        def load_mb(bb):
            mb_t = mbp.tile([P, S_TILES], f32)
            nc.gpsimd.dma_start(mb_t[:], mb[bb])
            mb_tiles[bb] = mb_t

        load_mb(0)
        if BL > 1:
            load_mb(1)

        row_state = {}  # b -> (u_ps, z_ps)
        COPY = mybir.ActivationFunctionType.Copy
        uz_queue = []     # stage-B work: (b, t0, h_se_b, p_b)
        drain_queue = []  # row drains, delayed >= 1 batch so the scalar
                          # copies never wait on in-flight U matmuls

        def emit_drains():
            while drain_queue:
                b_, u_ps, z_ps = drain_queue.pop(0)
                u_sb = uzp.tile([P, E], f32, tag="u_sb")
                z_sb = uzp.tile([H, 1], f32, tag="z_sb")
                nc.scalar.activation(u_sb[:], u_ps[:], COPY)
                nc.scalar.activation(z_sb[:], z_ps[:, 0:1], COPY)
                nc.scalar.dma_start(u_out[b_], u_sb[:])
                nc.scalar.dma_start(z_out[b_], z_sb[:])

        def emit_uz(work):
            """Stage B: U/Z matmuls, one transpose batch behind stage A
            so the exps have a full batch period to complete."""
            b_, t0_, h8_b, p_b = work
            u_ps, z_ps = row_state[b_]
            # U as 2 fp8 DoubleRow matmuls (adjacent s-tiles paired into
            # the K dimension), both accumulating into the same region
            # (DR forbids non-zero destination col groups)
            for j in range(TB // 2):
                nc.tensor.matmul(
                    u_ps[0:H, :],
                    p_b[:, 2 * j : 2 * j + 2, 0:H],
                    h8_b[:, 2 * j : 2 * j + 2, :],
                    start=(t0_ == 0 and j == 0),
                    stop=(t0_ + TB == S_TILES and j == TB // 2 - 1),
                    skip_group_check=True,
                    perf_mode=DR,
                )
            # Z as 2 fp8 DoubleRow matmuls over the same p pairs
            for j in range(TB // 2):
                nc.tensor.matmul(
                    z_ps[:],
                    p_b[:, 2 * j : 2 * j + 2, 0:H],
                    ones_t[:, :, 0:2],
                    start=(t0_ == 0 and j == 0),
                    stop=(t0_ + TB == S_TILES and j == TB // 2 - 1),
                    skip_group_check=True,
                    perf_mode=DR,
                )
            if t0_ + TB == S_TILES:
                drain_queue.append((b_, u_ps, z_ps))
                del row_state[b_]

        def emit_tails(pend):
            """Stage A for one transpose batch: scores + exp; then stage
            B (U/Z) for the previous batch and any due row drains."""
            b_, t0_, h8_b, ht_b = pend
            emit_drains()
            if b_ not in row_state:
                u_ps = ps_u.tile([P, E], f32)
                z_ps = ps_z.tile([H, 2], f32)
                row_state[b_] = (u_ps, z_ps)
            mb_t = mb_tiles[b_]
            # p padded to 16 cols: the DoubleRow weights AP needs a
            # 16-byte-aligned stride on the pair axis
            p_b = pp.tile([P, TB, 16], fp8)
            ht_v = ht_b[:].bitcast(fp8).rearrange(
                "p g (s two) -> p g two s", two=2
            )  # [P, TB*2, 2, 128]; (g=2c+?, two=j) selects e = 256c+2*e2+j
            for tt in range(TB):
                t_ = t0_ + tt
                # scores[s,h] = sum_e h[s,e] wkq[e,h]
                sc_ps = ps_s.tile([P, H], f32)
                kk = 0
                for c in range(2):
                    for jj in range(2):
                        nc.tensor.matmul(
                            sc_ps[:],
                            ht_v[:, tt * 2 + c, jj, :],
                            wkq_sb[:, c, jj, :],
                            start=(kk == 0),
                            stop=(kk == 3),
                        )
                        kk += 1
                # p = exp(scores + maskbias)/16; maskbias = -ln16 kept,
                # -1e4 masked (the 1/16 keeps fp8 p well inside e4m3
                # range; U/Z is scale-invariant so the host is unchanged)
                nc.scalar.activation(
                    p_b[:, tt, 0:H], sc_ps[:], EXP, bias=mb_t[:, t_ : t_ + 1]
                )
            uz_queue.append((b_, t0_, h8_b, p_b))
            if len(uz_queue) > 1:
                emit_uz(uz_queue.pop(0))

        pending = []
        for ci, (b, kch) in enumerate(chunks):
            s0, sl = kch * SC, SC
            x_sb = bufq.pop(0)
            # prefetch EARLY in program order so the transfers get multiple
            # chunk-periods of lead time — but RAMPED (not a burst): DMAHW
            # completion lanes are shared 8-wide round-robin in emission
            # order, so a startup burst makes the first transposes' lanes
            # alias still-in-flight megabyte chunks (false dependencies
            # that stall the score matmuls ~7us per early batch)
            while next_load < len(chunks) and next_load <= min(
                ci + 9, 2 * ci + 3
            ):
                bufq.append(load_chunk(next_load))
                next_load += 1
            for j in range(sl // (TB * P)):
                # --- projection for TB s-tiles ---
                h8_b = h8p.tile([P, TB, E], fp8, tag="h8")
                for tt in range(TB):
                    ts = (j * TB + tt) * P
                    h_ps = ps_h.tile([P, E], f32)
                    for c in range(C4):
                        nc.tensor.matmul(
                            h_ps[:],
                            x_sb[:, c, :, ts : ts + P],
                            wp_sb[:, c, :, :],
                            start=(c == 0),
                            stop=(c == C4 - 1) and not has_bp,
                            perf_mode=DR,
                        )
                    if has_bp:
                        nc.tensor.matmul(
                            h_ps[:], ones_row[:], bp_sb[:], start=False, stop=True
                        )
                    # relu + 1/256 scale in ONE vector op, straight to
                    # fp8: h8 = h_true/4 is the ONLY materialization of h —
                    # it feeds the U matmuls, and (bitcast to u16 pairs) the
                    # XBAR transpose for the score matmuls
                    nc.vector.tensor_scalar(
                        h8_b[:, tt, :], h_ps[:], 0.0, 1.0 / 256.0,
                        mybir.AluOpType.max, mybir.AluOpType.mult,
                    )
                # --- one batched SBUF->SBUF XBAR transpose for TB tiles.
                # Sync queue only: concurrent transposes from both HWDGE
                # queues race on the shared XBAR and corrupt data
                # (observed as nondeterministic output error) ---
                # one u16-viewed XBAR transpose per batch: adjacent
                # fp8 e-pairs ride as single 2-byte elements (the XBAR
                # does not support 1-byte dtypes), halving transpose bytes
                # vs a bf16 h. The score matmuls undo the pairing with
                # stride-2 stationary APs + host-deinterleaved wkq.
                ht_b = htp.tile([P, TB * 2, P], u16, tag="ht")
                nc.sync.dma_start_transpose(
                    ht_b[:], h8_b[:].bitcast(u16)
                )
                pending.append((b, (s0 // P) + j * TB, h8_b, ht_b))
                if len(pending) > 4:
                    emit_tails(pending.pop(0))
            if s0 == 0 and b + 2 < BL:
                load_mb(b + 2)
        while pending:
            emit_tails(pending.pop(0))
        while uz_queue:
            emit_uz(uz_queue.pop(0))
        emit_drains()

    nc.compile()
    with _nc_lock:
        _nc_cache[key] = nc
    return nc


def prepare_core_inputs(x, mask, Wp8, wkq8, bp=None):
    """Host-side packing for ONE core's shard."""
    import ml_dtypes

    fp8 = ml_dtypes.float8_e4m3
    BL_, S_, I_ = x.shape
    C4 = I_ // 256
    SC = 1024
    # xt[b, k, ki, c, ko, s] = x[b, SC*k + s, 256c + 128ko + ki]
    # (chunk-contiguous in DRAM: one chunk = one sequential 1MB block)
    x8 = x.astype(fp8)
    xt = np.ascontiguousarray(
        x8.reshape(BL_, S_ // SC, SC, C4, 2, P).transpose(0, 1, 5, 3, 4, 2)
    )
    # additive mask bias packed [BL, P, S_TILES]: -ln16 where kept (keeps
    # fp8 p inside e4m3 range; U/Z is invariant to the scale), -1e4 where
    # masked (exp underflows to exactly 0)
    mb = np.ascontiguousarray(
        ((mask.astype(np.float32) - 1.0) * 1.0e4 - np.log(16.0))
        .reshape(BL_, S_ // P, P)
        .transpose(0, 2, 1)
    ).astype(np.float32)
    m = {"xt": xt, "wp": Wp8, "wkq": wkq8, "mb": mb}
    if bp is not None:
        import ml_dtypes as md

        m["bp"] = (np.asarray(bp) * WP_SCALE).astype(md.bfloat16).reshape(1, E)
    return m


def kernel(
    x, mask, query, Wp, bp, Wq, bq, Wk, bk, Wv, bv, Wo, bo, W2, b2, gamma, beta,
    _trace=False,
):
    import ml_dtypes

    x = np.asarray(x)
    mask = np.asarray(mask)
    BL = B // NCORES

    # Host-side folds (all tiny)
    qh = (np.asarray(query, np.float64) @ np.asarray(Wq, np.float64)
          + np.asarray(bq, np.float64)).reshape(H, D)
    # h8 on-chip is at (1/4) the true h scale, so the folded score
    # weights carry a x4 factor; packed fp8, deinterleaved to match the
    # u16 pair-transpose: wkq8[c, j, p, :] = wkq[256c + 2p + j, :]
    wkq_scaled = np.einsum(
        "ehd,hd->eh",
        np.asarray(Wk, np.float64).reshape(E, H, D),
        qh,
    ) / np.sqrt(D) * 4.0
    wkq8 = np.ascontiguousarray(
        wkq_scaled.astype(np.float32)
        .reshape(2, 128, 2, H)
        .transpose(0, 2, 1, 3)
    ).astype(ml_dtypes.float8_e4m3)
    C4 = IN_DIM // 256
    Wp8 = np.ascontiguousarray(
        (np.asarray(Wp, np.float32) * WP_SCALE)
        .reshape(C4, 2, P, E)
        .transpose(2, 0, 1, 3)
    ).astype(ml_dtypes.float8_e4m3)

    has_bp = bool(np.any(np.asarray(bp)))
    nc = build_nc(has_bp=has_bp)

    in_maps = []
    for c in range(NCORES):
        sl = slice(c * BL, (c + 1) * BL)
        in_maps.append(
            prepare_core_inputs(
                x[sl], mask[sl], Wp8, wkq8,
                bp=np.asarray(bp) if has_bp else None,
            )
        )

    res = run_bass_kernel_spmd(
        nc, in_maps, core_ids=list(range(NCORES)), trace=_trace
    )
    u_raw = np.concatenate([r["u_out"] for r in res.results], axis=0)  # (B, P, E)
    z_raw = np.concatenate([r["z_out"] for r in res.results], axis=0)  # (B, H, 1)
    U = u_raw[:, 0:H, :].astype(np.float64)  # (B, H, E); rest is garbage
    Z = z_raw.astype(np.float64)  # (B, H, 1)

    # Host epilogue in float64 (the fp8 U path carries h at 1/4 true scale)
    pooledH = U / (Z * 0.25)  # (B, H, E)
    Wv64 = np.asarray(Wv, np.float64).reshape(E, H, D)
    pooled = np.einsum("bhe,ehd->bhd", pooledH, Wv64).reshape(B, E)
    pooled += np.asarray(bv, np.float64)
    pooled = pooled @ np.asarray(Wo, np.float64) + np.asarray(bo, np.float64)
    out = pooled @ np.asarray(W2, np.float64) + np.asarray(b2, np.float64)
    mu = out.mean(-1, keepdims=True)
    var = out.var(-1, keepdims=True)
    out = (out - mu) / np.sqrt(var + 1e-5) * np.asarray(gamma, np.float64) + np.asarray(
        beta, np.float64
    )
    out_f32 = out.astype(np.float32)
    if _trace:
        return out_f32, res
    return out_f32

